# revision 2
# baseline (speedup 1.0000x reference)
"""MoE (MiMoV2 FlashMoE) Trainium2 kernel: expert-parallel over 8 NeuronCores.

Strategy:
  Phase 1 (device): router — logits = x @ w_router.T computed in fp32,
    top-4 selection via exact max/mask iterations on logits, combine
    weights = sigmoid(logit) normalized over the selected 4. Each core
    handles T/8 = 512 tokens. Output: dense combine matrix [T, E]
    (4 nonzeros per row).
  Host: compaction — per-expert token index lists from combine > 0
    (pure data movement), gather token columns into per-expert capacity-C
    buffers (transposed layout [H, C]).
  Phase 2 (device): experts — 4 experts per core. For each expert:
    G^T = Wg @ Xg^T, U^T = Wu @ Xg^T, H = silu(G)*U*combine,
    Y^T = Wd @ H. All matmuls in float32r (TF32-like, full PE rate,
    ~1.5e-4 relative error). Output y^T [H, C] per expert, weighted.
  Host: scatter-add per-expert outputs into y [T, H] (unique indices
    per expert, ascending expert order matches reference accumulation).
"""
import math
import numpy as np
from contextlib import ExitStack

import concourse.bass as bass
import concourse.mybir as mybir
import concourse.tile as tile
from concourse import bacc
from concourse.bass_utils import run_bass_kernel_spmd

F32 = mybir.dt.float32
F32R = mybir.dt.float32r

# Problem shapes (hardcoded per contract)
E = 32          # experts
TOPK = 4
H = 1024        # hidden
I = 768         # intermediate
B, S = 2, 2048
T = B * S       # 4096 tokens
NCORES = 8
EPC = E // NCORES    # experts per core = 4
TPC = T // NCORES    # router tokens per core = 512
KH = H // 128        # 8 contraction chunks over H
KI = I // 128        # 6 contraction chunks over I

_program_cache = {}


def _ctiles(C):
    """Split C into near-equal tiles, each <= 512 (PSUM bank) and >= 256
    (float32r full-rate threshold) whenever C >= 512."""
    n = max(1, math.ceil(C / 512))
    base = C // n
    rem = C - base * n
    sizes = [base + (1 if i < rem else 0) for i in range(n)]
    out, off = [], 0
    for s in sizes:
        out.append((off, s))
        off += s
    return out


def build_router(reps=1):
    nc = bacc.Bacc()
    xTc = nc.dram_tensor("xTc", [H, TPC], F32, kind="ExternalInput")
    wrT = nc.dram_tensor("wrT", [H, E], F32, kind="ExternalInput")
    comb_out = nc.dram_tensor("comb", [TPC // 128, 128, E], F32,
                              kind="ExternalOutput")
    with ExitStack() as ctx:
        tc = ctx.enter_context(tile.TileContext(nc))
        sb = ctx.enter_context(tc.tile_pool(name="sb", bufs=1))
        work = ctx.enter_context(tc.tile_pool(name="work", bufs=2))
        ps = ctx.enter_context(tc.tile_pool(name="ps", bufs=2, space="PSUM"))

        xr = sb.tile([128, KH, TPC], F32)
        wr = sb.tile([128, KH, E], F32)
        for k in range(KH):
            nc.sync.dma_start(out=xr[:, k, :], in_=xTc[k * 128:(k + 1) * 128, :])
            nc.sync.dma_start(out=wr[:, k, :], in_=wrT[k * 128:(k + 1) * 128, :])

        for _ in range(reps):
            for t in range(TPC // 128):
                lp = ps.tile([128, E], F32)
                for k in range(KH):
                    nc.tensor.matmul(lp, xr[:, k, t * 128:(t + 1) * 128],
                                     wr[:, k, :], start=(k == 0), stop=(k == KH - 1))
                lt = work.tile([128, E], F32)
                nc.vector.tensor_copy(lt, lp)
                # top-4 selection on exact fp32 logits
                cur = work.tile([128, E], F32)
                nc.vector.tensor_copy(cur, lt)
                sel = work.tile([128, E], F32)
                nc.vector.memset(sel, 0.0)
                m = work.tile([128, 1], F32)
                iseq = work.tile([128, E], F32)
                for _k in range(TOPK):
                    nc.vector.reduce_max(m, cur, axis=mybir.AxisListType.X)
                    nc.vector.tensor_scalar(iseq, cur, m, None,
                                            op0=mybir.AluOpType.is_equal)
                    nc.vector.tensor_add(sel, sel, iseq)
                    nc.vector.scalar_tensor_tensor(cur, iseq, -1e30, cur,
                                                   op0=mybir.AluOpType.mult,
                                                   op1=mybir.AluOpType.add)
                sig = work.tile([128, E], F32)
                nc.scalar.activation(sig, lt, mybir.ActivationFunctionType.Sigmoid)
                wsel = work.tile([128, E], F32)
                nc.vector.tensor_mul(wsel, sel, sig)
                ssum = work.tile([128, 1], F32)
                nc.vector.reduce_sum(ssum, wsel, axis=mybir.AxisListType.X)
                nc.vector.tensor_scalar_add(ssum, ssum, 1e-20)
                rsum = work.tile([128, 1], F32)
                nc.vector.reciprocal(rsum, ssum)
                ct = work.tile([128, E], F32)
                nc.vector.tensor_scalar(ct, wsel, rsum, None,
                                        op0=mybir.AluOpType.mult)
                nc.sync.dma_start(out=comb_out[t], in_=ct)
    nc.finalize()
    return nc


def build_experts(C, reps=1):
    """Expert MLP kernel. Per-core inputs:
      xg [EPC, H, C] f32   gathered tokens, transposed
      wg [EPC, H, I] f32   w_gate[e].T
      wu [EPC, H, I] f32   w_up[e].T
      wd [EPC, I, H] f32   w_down[e].T
      cw [EPC, C]    f32   combine weights (0 on padding)
    Output: yg [EPC, H, C] f32 (already combine-weighted, transposed)."""
    nc = bacc.Bacc()
    xg = nc.dram_tensor("xg", [EPC, H, C], F32, kind="ExternalInput")
    wg = nc.dram_tensor("wg", [EPC, H, I], F32, kind="ExternalInput")
    wu = nc.dram_tensor("wu", [EPC, H, I], F32, kind="ExternalInput")
    wd = nc.dram_tensor("wd", [EPC, I, H], F32, kind="ExternalInput")
    cw = nc.dram_tensor("cw", [EPC, C], F32, kind="ExternalInput")
    yg = nc.dram_tensor("yg", [EPC, KH, 128, C], F32, kind="ExternalOutput")

    cts = _ctiles(C)
    with ExitStack() as ctx:
        tc = ctx.enter_context(tile.TileContext(nc))
        cwp = ctx.enter_context(tc.tile_pool(name="cwp", bufs=1))
        xgp = ctx.enter_context(tc.tile_pool(name="xgp", bufs=2))
        wgup = ctx.enter_context(tc.tile_pool(name="wgup", bufs=3))
        wdp = ctx.enter_context(tc.tile_pool(name="wdp", bufs=3))
        hp = ctx.enter_context(tc.tile_pool(name="hp", bufs=2))
        msc = ctx.enter_context(tc.tile_pool(name="msc", bufs=4))
        outp = ctx.enter_context(tc.tile_pool(name="outp", bufs=4))
        ps_gu = ctx.enter_context(tc.tile_pool(name="ps_gu", bufs=2, space="PSUM"))
        ps_d = ctx.enter_context(tc.tile_pool(name="ps_d", bufs=2, space="PSUM"))

        cwb = []
        for j in range(EPC):
            cwt = cwp.tile([128, C], F32, tag=f"cw{j}")
            nc.gpsimd.dma_start(out=cwt, in_=cw[j:j + 1, :].partition_broadcast(128))
            cwb.append(cwt)

        for _ in range(reps):
            for j in range(EPC):
                xg_t = xgp.tile([128, KH, C], F32R)
                for k in range(KH):
                    nc.gpsimd.dma_start(out=xg_t[:, k, :],
                                        in_=xg[j, k * 128:(k + 1) * 128, :])
                h_t = hp.tile([128, KI, C], F32R)
                for m in range(KI):
                    wgu_t = wgup.tile([128, KH, 2, 128], F32R)
                    nc.gpsimd.dma_start(
                        out=wgu_t[:, :, 0, :],
                        in_=wg[j, :, m * 128:(m + 1) * 128]
                        .rearrange("(k p) i -> p k i", p=128))
                    nc.gpsimd.dma_start(
                        out=wgu_t[:, :, 1, :],
                        in_=wu[j, :, m * 128:(m + 1) * 128]
                        .rearrange("(k p) i -> p k i", p=128))
                    for (c0, cn) in cts:
                        gp = ps_gu.tile([128, cn], F32, tag="gp")
                        for k in range(KH):
                            nc.tensor.matmul(gp, wgu_t[:, k, 0, :],
                                             xg_t[:, k, c0:c0 + cn],
                                             start=(k == 0), stop=(k == KH - 1))
                        up = ps_gu.tile([128, cn], F32, tag="up")
                        for k in range(KH):
                            nc.tensor.matmul(up, wgu_t[:, k, 1, :],
                                             xg_t[:, k, c0:c0 + cn],
                                             start=(k == 0), stop=(k == KH - 1))
                        sg = msc.tile([128, cn], F32, tag="sg")
                        nc.scalar.activation(sg, gp,
                                             mybir.ActivationFunctionType.Silu)
                        t1 = msc.tile([128, cn], F32, tag="t1")
                        nc.vector.tensor_mul(t1, sg, up)
                        nc.vector.tensor_mul(h_t[:, m, c0:c0 + cn], t1,
                                             cwb[j][:, c0:c0 + cn])
                for h in range(KH):
                    wd_t = wdp.tile([128, KI, 128], F32R)
                    nc.gpsimd.dma_start(
                        out=wd_t,
                        in_=wd[j, :, h * 128:(h + 1) * 128]
                        .rearrange("(k p) o -> p k o", p=128))
                    for (c0, cn) in cts:
                        yp = ps_d.tile([128, cn], F32, tag="yp")
                        for k in range(KI):
                            nc.tensor.matmul(yp, wd_t[:, k, :],
                                             h_t[:, k, c0:c0 + cn],
                                             start=(k == 0), stop=(k == KI - 1))
                        yo = outp.tile([128, cn], F32, tag="yo")
                        nc.scalar.copy(yo, yp)
                        nc.sync.dma_start(out=yg[j, h, :, c0:c0 + cn], in_=yo)
    nc.finalize()
    return nc


def _get_router():
    if "router" not in _program_cache:
        _program_cache["router"] = build_router()
    return _program_cache["router"]


def _get_experts(C):
    key = ("experts", C)
    if key not in _program_cache:
        _program_cache[key] = build_experts(C)
    return _program_cache[key]


def kernel(hidden_states, w_router, w_gate, w_up, w_down):
    x = np.ascontiguousarray(np.asarray(hidden_states, np.float32)).reshape(T, H)
    xT = np.ascontiguousarray(x.T)                                   # [H, T]
    wrT = np.ascontiguousarray(np.asarray(w_router, np.float32).T)   # [H, E]

    # ---- Phase 1: router on device ----
    nc1 = _get_router()
    in_maps1 = [
        {"xTc": np.ascontiguousarray(xT[:, c * TPC:(c + 1) * TPC]), "wrT": wrT}
        for c in range(NCORES)
    ]
    r1 = run_bass_kernel_spmd(nc1, in_maps1, list(range(NCORES)))
    combine = np.concatenate(
        [r1.results[c]["comb"].reshape(TPC, E) for c in range(NCORES)], axis=0)

    # ---- Host: compaction (data movement only) ----
    idx = [np.nonzero(combine[:, e])[0] for e in range(E)]
    maxn = max(len(ii) for ii in idx)
    C = max(512, ((maxn + 63) // 64) * 64)

    wgT = np.ascontiguousarray(np.asarray(w_gate, np.float32).transpose(0, 2, 1))
    wuT = np.ascontiguousarray(np.asarray(w_up, np.float32).transpose(0, 2, 1))
    wdT = np.ascontiguousarray(np.asarray(w_down, np.float32).transpose(0, 2, 1))

    in_maps2 = []
    for c in range(NCORES):
        xg = np.zeros((EPC, H, C), np.float32)
        cwm = np.zeros((EPC, C), np.float32)
        for j in range(EPC):
            e = c * EPC + j
            ii = idx[e]
            n = len(ii)
            if n:
                xg[j, :, :n] = xT[:, ii]
                cwm[j, :n] = combine[ii, e]
        in_maps2.append({
            "xg": xg,
            "wg": wgT[c * EPC:(c + 1) * EPC],
            "wu": wuT[c * EPC:(c + 1) * EPC],
            "wd": wdT[c * EPC:(c + 1) * EPC],
            "cw": cwm,
        })

    # ---- Phase 2: expert MLPs on device ----
    nc2 = _get_experts(C)
    r2 = run_bass_kernel_spmd(nc2, in_maps2, list(range(NCORES)))

    # ---- Host: scatter-add (unique indices per expert) ----
    y = np.zeros((T, H), np.float32)
    for c in range(NCORES):
        ygc = r2.results[c]["yg"].reshape(EPC, H, C)
        for j in range(EPC):
            e = c * EPC + j
            ii = idx[e]
            n = len(ii)
            if n:
                y[ii] += ygc[j, :, :n].T
    return y.reshape(B, S, H)


# revision 3
# speedup vs baseline: 2.2363x; 2.2363x over previous
"""MoE (MiMoV2 FlashMoE) Trainium2 kernel: expert-parallel over 8 NeuronCores.

Strategy:
  Phase 1 (device): router — logits = x @ w_router.T computed in fp32,
    top-4 selection via exact max/mask iterations on logits, combine
    weights = sigmoid(logit) normalized over the selected 4. Each core
    handles T/8 = 512 tokens. Output: dense combine matrix [T, E]
    (4 nonzeros per row).
  Host: compaction — per-expert token index lists from combine > 0
    (pure data movement), gather token columns into per-expert capacity-C
    buffers laid out exactly as the SBUF tiles (contiguous DMA).
  Phase 2 (device): experts — 4 experts per core. For each expert:
    G^T = Wg @ Xg^T, U^T = Wu @ Xg^T, Hm = silu(G)*U*combine,
    Y^T = Wd @ Hm. Matmuls in float32r (TF32-like, full PE rate,
    ~1.5e-4 relative error). Output y^T [H, C] per expert, weighted.
  Host: scatter-add per-expert outputs into y [T, H] (unique indices
    per expert, ascending expert order matches reference accumulation).
"""
import math
import numpy as np
from contextlib import ExitStack

import concourse.bass as bass
import concourse.mybir as mybir
import concourse.tile as tile
from concourse import bacc
from concourse.bass_utils import run_bass_kernel_spmd

F32 = mybir.dt.float32
F32R = mybir.dt.float32r

# Problem shapes (hardcoded per contract)
E = 32          # experts
TOPK = 4
H = 1024        # hidden
I = 768         # intermediate
B, S = 2, 2048
T = B * S       # 4096 tokens
NCORES = 8
EPC = E // NCORES    # experts per core = 4
TPC = T // NCORES    # router tokens per core = 512
KH = H // 128        # 8 contraction chunks over H
KI = I // 128        # 6 contraction chunks over I

_program_cache = {}


def _ctiles(C):
    """Split C into near-equal tiles, each <= 512 (PSUM bank) and >= 256
    (float32r full-rate threshold) whenever C >= 512."""
    n = max(1, math.ceil(C / 512))
    base = C // n
    rem = C - base * n
    sizes = [base + (1 if i < rem else 0) for i in range(n)]
    out, off = [], 0
    for s in sizes:
        out.append((off, s))
        off += s
    return out


def build_router(reps=1):
    nc = bacc.Bacc()
    xTc = nc.dram_tensor("xTc", [H, TPC], F32, kind="ExternalInput")
    wrT = nc.dram_tensor("wrT", [H, E], F32, kind="ExternalInput")
    comb_out = nc.dram_tensor("comb", [TPC // 128, 128, E], F32,
                              kind="ExternalOutput")
    with ExitStack() as ctx:
        tc = ctx.enter_context(tile.TileContext(nc))
        sb = ctx.enter_context(tc.tile_pool(name="sb", bufs=1))
        work = ctx.enter_context(tc.tile_pool(name="work", bufs=2))
        ps = ctx.enter_context(tc.tile_pool(name="ps", bufs=2, space="PSUM"))

        xr = sb.tile([128, KH, TPC], F32)
        wr = sb.tile([128, KH, E], F32)
        for k in range(KH):
            nc.sync.dma_start(out=xr[:, k, :], in_=xTc[k * 128:(k + 1) * 128, :])
            nc.sync.dma_start(out=wr[:, k, :], in_=wrT[k * 128:(k + 1) * 128, :])

        for _ in range(reps):
            for t in range(TPC // 128):
                lp = ps.tile([128, E], F32)
                for k in range(KH):
                    nc.tensor.matmul(lp, xr[:, k, t * 128:(t + 1) * 128],
                                     wr[:, k, :], start=(k == 0), stop=(k == KH - 1))
                lt = work.tile([128, E], F32)
                nc.vector.tensor_copy(lt, lp)
                # top-4 selection on exact fp32 logits
                cur = work.tile([128, E], F32)
                nc.vector.tensor_copy(cur, lt)
                sel = work.tile([128, E], F32)
                nc.vector.memset(sel, 0.0)
                m = work.tile([128, 1], F32)
                iseq = work.tile([128, E], F32)
                for _k in range(TOPK):
                    nc.vector.reduce_max(m, cur, axis=mybir.AxisListType.X)
                    nc.vector.tensor_scalar(iseq, cur, m, None,
                                            op0=mybir.AluOpType.is_equal)
                    nc.vector.tensor_add(sel, sel, iseq)
                    nc.vector.scalar_tensor_tensor(cur, iseq, -1e30, cur,
                                                   op0=mybir.AluOpType.mult,
                                                   op1=mybir.AluOpType.add)
                sig = work.tile([128, E], F32)
                nc.scalar.activation(sig, lt, mybir.ActivationFunctionType.Sigmoid)
                wsel = work.tile([128, E], F32)
                nc.vector.tensor_mul(wsel, sel, sig)
                ssum = work.tile([128, 1], F32)
                nc.vector.reduce_sum(ssum, wsel, axis=mybir.AxisListType.X)
                nc.vector.tensor_scalar_add(ssum, ssum, 1e-20)
                rsum = work.tile([128, 1], F32)
                nc.vector.reciprocal(rsum, ssum)
                ct = work.tile([128, E], F32)
                nc.vector.tensor_scalar(ct, wsel, rsum, None,
                                        op0=mybir.AluOpType.mult)
                nc.sync.dma_start(out=comb_out[t], in_=ct)
    nc.finalize()
    return nc


def build_experts(C, reps=1):
    """Expert MLP kernel. Per-core inputs (pre-laid-out for SBUF tiles):
      xg  [EPC, 128, KH, C]        f32r  xg[j,p,k,c] = x[tok_c, k*128+p]
      wgu [EPC, KI, 128, KH, 2, 128] f32r  [...,0,i]=w_gate[e,m*128+i,k*128+p]
      wd  [EPC, KH, 128, KI, 128]  f32r  wd[j,h,p,k,o]=w_down[e,h*128+o,k*128+p]
      cw  [EPC, C]                 f32   combine weights (0 on padding)
    Output: yg [EPC, KH, 128, C] f32 (combine-weighted, transposed)."""
    nc = bacc.Bacc()
    xg = nc.dram_tensor("xg", [EPC, 128, KH, C], F32R, kind="ExternalInput")
    wgu = nc.dram_tensor("wgu", [EPC, KI, 128, KH, 2, 128], F32R,
                         kind="ExternalInput")
    wd = nc.dram_tensor("wd", [EPC, KH, 128, KI, 128], F32R,
                        kind="ExternalInput")
    cw = nc.dram_tensor("cw", [EPC, C], F32, kind="ExternalInput")
    yg = nc.dram_tensor("yg", [EPC, KH, 128, C], F32, kind="ExternalOutput")

    cts = _ctiles(C)
    with ExitStack() as ctx:
        tc = ctx.enter_context(tile.TileContext(nc))
        cwp = ctx.enter_context(tc.tile_pool(name="cwp", bufs=1))
        xgp = ctx.enter_context(tc.tile_pool(name="xgp", bufs=2))
        wgup = ctx.enter_context(tc.tile_pool(name="wgup", bufs=3))
        wdp = ctx.enter_context(tc.tile_pool(name="wdp", bufs=3))
        hp = ctx.enter_context(tc.tile_pool(name="hp", bufs=2))
        msc = ctx.enter_context(tc.tile_pool(name="msc", bufs=4))
        outp = ctx.enter_context(tc.tile_pool(name="outp", bufs=4))
        ps_gu = ctx.enter_context(tc.tile_pool(name="ps_gu", bufs=2, space="PSUM"))
        ps_d = ctx.enter_context(tc.tile_pool(name="ps_d", bufs=2, space="PSUM"))

        cwb = []
        for j in range(EPC):
            cwt = cwp.tile([128, C], F32, tag=f"cw{j}")
            nc.gpsimd.dma_start(out=cwt, in_=cw[j:j + 1, :].partition_broadcast(128))
            cwb.append(cwt)

        for _ in range(reps):
            for j in range(EPC):
                xg_t = xgp.tile([128, KH, C], F32R)
                nc.sync.dma_start(out=xg_t, in_=xg[j])
                h_t = hp.tile([128, KI, C], F32R)
                for m in range(KI):
                    wgu_t = wgup.tile([128, KH, 2, 128], F32R)
                    nc.sync.dma_start(out=wgu_t, in_=wgu[j, m])
                    for (c0, cn) in cts:
                        gp = ps_gu.tile([128, cn], F32, tag="gp")
                        for k in range(KH):
                            nc.tensor.matmul(gp, wgu_t[:, k, 0, :],
                                             xg_t[:, k, c0:c0 + cn],
                                             start=(k == 0), stop=(k == KH - 1))
                        up = ps_gu.tile([128, cn], F32, tag="up")
                        for k in range(KH):
                            nc.tensor.matmul(up, wgu_t[:, k, 1, :],
                                             xg_t[:, k, c0:c0 + cn],
                                             start=(k == 0), stop=(k == KH - 1))
                        sg = msc.tile([128, cn], F32, tag="sg")
                        nc.scalar.activation(sg, gp,
                                             mybir.ActivationFunctionType.Silu)
                        t1 = msc.tile([128, cn], F32, tag="t1")
                        nc.vector.tensor_mul(t1, sg, up)
                        nc.vector.tensor_mul(h_t[:, m, c0:c0 + cn], t1,
                                             cwb[j][:, c0:c0 + cn])
                for h in range(KH):
                    wd_t = wdp.tile([128, KI, 128], F32R)
                    nc.sync.dma_start(out=wd_t, in_=wd[j, h])
                    for (c0, cn) in cts:
                        yp = ps_d.tile([128, cn], F32, tag="yp")
                        for k in range(KI):
                            nc.tensor.matmul(yp, wd_t[:, k, :],
                                             h_t[:, k, c0:c0 + cn],
                                             start=(k == 0), stop=(k == KI - 1))
                        yo = outp.tile([128, cn], F32, tag="yo")
                        nc.scalar.copy(yo, yp)
                        nc.sync.dma_start(out=yg[j, h, :, c0:c0 + cn], in_=yo)
    nc.finalize()
    return nc


def _get_router():
    if "router" not in _program_cache:
        _program_cache["router"] = build_router()
    return _program_cache["router"]


def _get_experts(C):
    key = ("experts", C)
    if key not in _program_cache:
        _program_cache[key] = build_experts(C)
    return _program_cache[key]


def prep_router_inputs(x):
    xT = np.ascontiguousarray(x.T)
    return xT


def route_on_host(combine):
    idx = [np.nonzero(combine[:, e])[0] for e in range(E)]
    maxn = max(len(ii) for ii in idx)
    C = max(512, ((maxn + 63) // 64) * 64)
    return idx, C


def prep_expert_inputs(x, combine, idx, C, w_gate, w_up, w_down):
    """Build per-core in_maps with tile-exact layouts (all contiguous DMA)."""
    in_maps = []
    for c in range(NCORES):
        xg = np.zeros((EPC, 128, KH, C), np.float32)
        cwm = np.zeros((EPC, C), np.float32)
        wgu = np.empty((EPC, KI, 128, KH, 2, 128), np.float32)
        wdh = np.empty((EPC, KH, 128, KI, 128), np.float32)
        for j in range(EPC):
            e = c * EPC + j
            ii = idx[e]
            n = len(ii)
            if n:
                # [n, H] -> [n, KH, 128] -> [128, KH, n]
                xe = x[ii].reshape(n, KH, 128).transpose(2, 1, 0)
                xg[j, :, :, :n] = xe
                cwm[j, :n] = combine[ii, e]
            g = w_gate[e].reshape(KI, 128, KH, 128)   # (m, i, k, p)
            u = w_up[e].reshape(KI, 128, KH, 128)
            wgu[j, :, :, :, 0, :] = g.transpose(0, 3, 2, 1)   # (m, p, k, i)
            wgu[j, :, :, :, 1, :] = u.transpose(0, 3, 2, 1)
            d = w_down[e].reshape(KH, 128, KI, 128)   # (h, o, k, p)
            wdh[j] = d.transpose(0, 3, 2, 1)          # (h, p, k, o)
        in_maps.append({"xg": xg, "wgu": wgu, "wd": wdh, "cw": cwm})
    return in_maps


def kernel(hidden_states, w_router, w_gate, w_up, w_down):
    x = np.ascontiguousarray(np.asarray(hidden_states, np.float32)).reshape(T, H)
    w_gate = np.asarray(w_gate, np.float32)
    w_up = np.asarray(w_up, np.float32)
    w_down = np.asarray(w_down, np.float32)
    xT = prep_router_inputs(x)
    wrT = np.ascontiguousarray(np.asarray(w_router, np.float32).T)   # [H, E]

    # ---- Phase 1: router on device ----
    nc1 = _get_router()
    in_maps1 = [
        {"xTc": np.ascontiguousarray(xT[:, c * TPC:(c + 1) * TPC]), "wrT": wrT}
        for c in range(NCORES)
    ]
    r1 = run_bass_kernel_spmd(nc1, in_maps1, list(range(NCORES)))
    combine = np.concatenate(
        [r1.results[c]["comb"].reshape(TPC, E) for c in range(NCORES)], axis=0)

    # ---- Host: compaction (data movement only) ----
    idx, C = route_on_host(combine)
    in_maps2 = prep_expert_inputs(x, combine, idx, C, w_gate, w_up, w_down)

    # ---- Phase 2: expert MLPs on device ----
    nc2 = _get_experts(C)
    r2 = run_bass_kernel_spmd(nc2, in_maps2, list(range(NCORES)))

    # ---- Host: scatter-add (unique indices per expert) ----
    y = np.zeros((T, H), np.float32)
    for c in range(NCORES):
        ygc = r2.results[c]["yg"].reshape(EPC, H, C)
        for j in range(EPC):
            e = c * EPC + j
            ii = idx[e]
            n = len(ii)
            if n:
                y[ii] += ygc[j, :, :n].T
    return y.reshape(B, S, H)


# revision 7
# speedup vs baseline: 9.3421x; 4.1776x over previous
"""MoE (MiMoV2 FlashMoE) Trainium2 kernel: expert-parallel over 8 NeuronCores.

Strategy:
  Phase 1 (device): router — logits = x @ w_router.T computed in fp32,
    top-4 selection via exact max/mask iterations on logits, combine
    weights = sigmoid(logit) normalized over the selected 4. Each core
    handles T/8 = 512 tokens. Output: dense combine matrix [T, E]
    (4 nonzeros per row).
  Host: compaction — per-expert token index lists from combine > 0
    (pure data movement), gather token columns into per-expert capacity-C
    buffers laid out exactly as the SBUF tiles (contiguous DMA).
  Phase 2 (device): experts — 4 experts per core. For each expert:
    G^T = Wg @ Xg^T, U^T = Wu @ Xg^T, Hm = silu(G)*U*combine,
    Y^T = Wd @ Hm. Matmuls in float32r (TF32-like, full PE rate,
    ~1.5e-4 relative error). Output y^T [H, C] per expert, weighted.
  Host: scatter-add per-expert outputs into y [T, H] (unique indices
    per expert, ascending expert order matches reference accumulation).
"""
import math
import numpy as np
from contextlib import ExitStack

import concourse.bass as bass
import concourse.mybir as mybir
import concourse.tile as tile
from concourse import bacc
from concourse.bass_utils import run_bass_kernel_spmd

F32 = mybir.dt.float32
F32R = mybir.dt.float32r

# Problem shapes (hardcoded per contract)
E = 32          # experts
TOPK = 4
H = 1024        # hidden
I = 768         # intermediate
B, S = 2, 2048
T = B * S       # 4096 tokens
NCORES = 8
EPC = E // NCORES    # experts per core = 4
TPC = T // NCORES    # router tokens per core = 512
KH = H // 128        # 8 contraction chunks over H
KI = I // 128        # 6 contraction chunks over I

_program_cache = {}


def _ctiles(C):
    """Split C into near-equal tiles, each <= 512 (PSUM bank) and >= 256
    (float32r full-rate threshold) whenever C >= 512."""
    n = max(1, math.ceil(C / 512))
    base = C // n
    rem = C - base * n
    sizes = [base + (1 if i < rem else 0) for i in range(n)]
    out, off = [], 0
    for s in sizes:
        out.append((off, s))
        off += s
    return out


def build_router(reps=1):
    nc = bacc.Bacc()
    xTc = nc.dram_tensor("xTc", [H, TPC], F32, kind="ExternalInput")
    wrT = nc.dram_tensor("wrT", [H, E], F32, kind="ExternalInput")
    comb_out = nc.dram_tensor("comb", [TPC // 128, 128, E], F32,
                              kind="ExternalOutput")
    with ExitStack() as ctx:
        tc = ctx.enter_context(tile.TileContext(nc))
        sb = ctx.enter_context(tc.tile_pool(name="sb", bufs=1))
        work = ctx.enter_context(tc.tile_pool(name="work", bufs=2))
        ps = ctx.enter_context(tc.tile_pool(name="ps", bufs=2, space="PSUM"))

        xr = sb.tile([128, KH, TPC], F32)
        wr = sb.tile([128, KH, E], F32)
        for k in range(KH):
            nc.sync.dma_start(out=xr[:, k, :], in_=xTc[k * 128:(k + 1) * 128, :])
            nc.sync.dma_start(out=wr[:, k, :], in_=wrT[k * 128:(k + 1) * 128, :])

        for _ in range(reps):
            for t in range(TPC // 128):
                lp = ps.tile([128, E], F32)
                for k in range(KH):
                    nc.tensor.matmul(lp, xr[:, k, t * 128:(t + 1) * 128],
                                     wr[:, k, :], start=(k == 0), stop=(k == KH - 1))
                lt = work.tile([128, E], F32)
                nc.vector.tensor_copy(lt, lp)
                # top-4 selection on exact fp32 logits
                cur = work.tile([128, E], F32)
                nc.vector.tensor_copy(cur, lt)
                sel = work.tile([128, E], F32)
                nc.vector.memset(sel, 0.0)
                m = work.tile([128, 1], F32)
                iseq = work.tile([128, E], F32)
                for _k in range(TOPK):
                    nc.vector.reduce_max(m, cur, axis=mybir.AxisListType.X)
                    nc.vector.tensor_scalar(iseq, cur, m, None,
                                            op0=mybir.AluOpType.is_equal)
                    nc.vector.tensor_add(sel, sel, iseq)
                    nc.vector.scalar_tensor_tensor(cur, iseq, -1e30, cur,
                                                   op0=mybir.AluOpType.mult,
                                                   op1=mybir.AluOpType.add)
                sig = work.tile([128, E], F32)
                nc.scalar.activation(sig, lt, mybir.ActivationFunctionType.Sigmoid)
                wsel = work.tile([128, E], F32)
                nc.vector.tensor_mul(wsel, sel, sig)
                ssum = work.tile([128, 1], F32)
                nc.vector.reduce_sum(ssum, wsel, axis=mybir.AxisListType.X)
                nc.vector.tensor_scalar_add(ssum, ssum, 1e-20)
                rsum = work.tile([128, 1], F32)
                nc.vector.reciprocal(rsum, ssum)
                ct = work.tile([128, E], F32)
                nc.vector.tensor_scalar(ct, wsel, rsum, None,
                                        op0=mybir.AluOpType.mult)
                nc.sync.dma_start(out=comb_out[t], in_=ct)
    nc.finalize()
    return nc


def build_experts(C, reps=1):
    """Expert MLP kernel. Per-core inputs (pre-laid-out for SBUF tiles):
      xg  [EPC, 128, KH, C]        f32r  xg[j,p,k,c] = x[tok_c, k*128+p]
      wgu [EPC, KI, 128, KH, 2, 128] f32r  [...,0,i]=w_gate[e,m*128+i,k*128+p]
      wd  [EPC, KH, 128, KI, 128]  f32r  wd[j,h,p,k,o]=w_down[e,h*128+o,k*128+p]
      cw  [EPC, C]                 f32   combine weights (0 on padding)
    Output: yg [EPC, 128, KH, C] f32, yg[j,p,h,c] = y^T[h*128+p, c]
    (combine-weighted, transposed)."""
    nc = bacc.Bacc()
    xg = nc.dram_tensor("xg", [EPC, 128, KH, C], F32R, kind="ExternalInput")
    wgu = nc.dram_tensor("wgu", [EPC, KI, 128, KH, 2, 128], F32R,
                         kind="ExternalInput")
    wd = nc.dram_tensor("wd", [EPC, KH, 128, KI, 128], F32R,
                        kind="ExternalInput")
    cw = nc.dram_tensor("cw", [EPC, C], F32, kind="ExternalInput")
    yg = nc.dram_tensor("yg", [EPC, 128, KH, C], F32, kind="ExternalOutput")

    cts = _ctiles(C)
    with ExitStack() as ctx:
        tc = ctx.enter_context(tile.TileContext(nc))
        cwp = ctx.enter_context(tc.tile_pool(name="cwp", bufs=1))
        xgp = ctx.enter_context(tc.tile_pool(name="xgp", bufs=2))
        wgup = ctx.enter_context(tc.tile_pool(name="wgup", bufs=3))
        wdp = ctx.enter_context(tc.tile_pool(name="wdp", bufs=3))
        hp = ctx.enter_context(tc.tile_pool(name="hp", bufs=2))
        msc = ctx.enter_context(tc.tile_pool(name="msc", bufs=4))
        outp = ctx.enter_context(tc.tile_pool(name="outp", bufs=2))
        ps_gu = ctx.enter_context(tc.tile_pool(name="ps_gu", bufs=2, space="PSUM"))
        ps_d = ctx.enter_context(tc.tile_pool(name="ps_d", bufs=2, space="PSUM"))

        cwb = []
        for j in range(EPC):
            cwt = cwp.tile([128, C], F32, tag=f"cw{j}")
            nc.gpsimd.dma_start(out=cwt, in_=cw[j:j + 1, :].partition_broadcast(128))
            cwb.append(cwt)

        for _ in range(reps):
            for j in range(EPC):
                xg_t = xgp.tile([128, KH, C], F32R)
                nc.sync.dma_start(out=xg_t, in_=xg[j])
                h_t = hp.tile([128, KI, C], F32R)
                for m in range(KI):
                    wgu_t = wgup.tile([128, KH, 2, 128], F32R)
                    nc.sync.dma_start(out=wgu_t, in_=wgu[j, m])
                    for (c0, cn) in cts:
                        gp = ps_gu.tile([128, cn], F32, tag="gp")
                        for k in range(KH):
                            nc.tensor.matmul(gp, wgu_t[:, k, 0, :],
                                             xg_t[:, k, c0:c0 + cn],
                                             start=(k == 0), stop=(k == KH - 1))
                        up = ps_gu.tile([128, cn], F32, tag="up")
                        for k in range(KH):
                            nc.tensor.matmul(up, wgu_t[:, k, 1, :],
                                             xg_t[:, k, c0:c0 + cn],
                                             start=(k == 0), stop=(k == KH - 1))
                        sg = msc.tile([128, cn], F32, tag="sg")
                        nc.scalar.activation(sg, gp,
                                             mybir.ActivationFunctionType.Silu)
                        t1 = msc.tile([128, cn], F32, tag="t1")
                        nc.vector.tensor_mul(t1, sg, up)
                        nc.vector.tensor_mul(h_t[:, m, c0:c0 + cn], t1,
                                             cwb[j][:, c0:c0 + cn])
                yo_all = outp.tile([128, KH, C], F32, tag="yo")
                for h in range(KH):
                    wd_t = wdp.tile([128, KI, 128], F32R)
                    nc.sync.dma_start(out=wd_t, in_=wd[j, h])
                    for (c0, cn) in cts:
                        yp = ps_d.tile([128, cn], F32, tag="yp")
                        for k in range(KI):
                            nc.tensor.matmul(yp, wd_t[:, k, :],
                                             h_t[:, k, c0:c0 + cn],
                                             start=(k == 0), stop=(k == KI - 1))
                        nc.vector.tensor_copy(yo_all[:, h, c0:c0 + cn], yp)
                nc.sync.dma_start(out=yg[j], in_=yo_all)
    nc.finalize()
    return nc


def _get_router():
    if "router" not in _program_cache:
        _program_cache["router"] = build_router()
    return _program_cache["router"]


def _get_experts(C):
    key = ("experts", C)
    if key not in _program_cache:
        _program_cache[key] = build_experts(C)
    return _program_cache[key]


def prep_router_inputs(x):
    xT = np.ascontiguousarray(x.T)
    return xT


def route_on_host(combine):
    idx = [np.nonzero(combine[:, e])[0] for e in range(E)]
    maxn = max(len(ii) for ii in idx)
    C = max(512, ((maxn + 63) // 64) * 64)
    return idx, C


def prep_expert_inputs(x, combine, idx, C, w_gate, w_up, w_down):
    """Build per-core in_maps with tile-exact layouts (all contiguous DMA)."""
    in_maps = []
    for c in range(NCORES):
        xg = np.zeros((EPC, 128, KH, C), np.float32)
        cwm = np.zeros((EPC, C), np.float32)
        wgu = np.empty((EPC, KI, 128, KH, 2, 128), np.float32)
        wdh = np.empty((EPC, KH, 128, KI, 128), np.float32)
        for j in range(EPC):
            e = c * EPC + j
            ii = idx[e]
            n = len(ii)
            if n:
                # [n, H] -> [n, KH, 128] -> [128, KH, n]
                xe = x[ii].reshape(n, KH, 128).transpose(2, 1, 0)
                xg[j, :, :, :n] = xe
                cwm[j, :n] = combine[ii, e]
            g = w_gate[e].reshape(KI, 128, KH, 128)   # (m, i, k, p)
            u = w_up[e].reshape(KI, 128, KH, 128)
            wgu[j, :, :, :, 0, :] = g.transpose(0, 3, 2, 1)   # (m, p, k, i)
            wgu[j, :, :, :, 1, :] = u.transpose(0, 3, 2, 1)
            d = w_down[e].reshape(KH, 128, KI, 128)   # (h, o, k, p)
            wdh[j] = d.transpose(0, 3, 2, 1)          # (h, p, k, o)
        in_maps.append({"xg": xg, "wgu": wgu, "wd": wdh, "cw": cwm})
    return in_maps


def kernel(hidden_states, w_router, w_gate, w_up, w_down):
    x = np.ascontiguousarray(np.asarray(hidden_states, np.float32)).reshape(T, H)
    w_gate = np.asarray(w_gate, np.float32)
    w_up = np.asarray(w_up, np.float32)
    w_down = np.asarray(w_down, np.float32)
    xT = prep_router_inputs(x)
    wrT = np.ascontiguousarray(np.asarray(w_router, np.float32).T)   # [H, E]

    # ---- Phase 1: router on device ----
    nc1 = _get_router()
    in_maps1 = [
        {"xTc": np.ascontiguousarray(xT[:, c * TPC:(c + 1) * TPC]), "wrT": wrT}
        for c in range(NCORES)
    ]
    r1 = run_bass_kernel_spmd(nc1, in_maps1, list(range(NCORES)))
    combine = np.concatenate(
        [r1.results[c]["comb"].reshape(TPC, E) for c in range(NCORES)], axis=0)

    # ---- Host: compaction (data movement only) ----
    idx, C = route_on_host(combine)
    in_maps2 = prep_expert_inputs(x, combine, idx, C, w_gate, w_up, w_down)

    # ---- Phase 2: expert MLPs on device ----
    nc2 = _get_experts(C)
    r2 = run_bass_kernel_spmd(nc2, in_maps2, list(range(NCORES)))

    # ---- Host: scatter-add (unique indices per expert) ----
    y = np.zeros((T, H), np.float32)
    for c in range(NCORES):
        ygc = r2.results[c]["yg"]          # [EPC, 128, KH, C]
        for j in range(EPC):
            e = c * EPC + j
            ii = idx[e]
            n = len(ii)
            if n:
                # [128(p), KH(h), C] -> [H, C]: H index = h*128 + p
                yt = ygc[j].transpose(1, 0, 2).reshape(H, C)
                y[ii] += yt[:, :n].T
    return y.reshape(B, S, H)
